# revision 1
# baseline (speedup 1.0000x reference)
"""Trainium2 Bass kernel for nn_DGraphAttention (gnn_message_passing).

Math (reference):
    x = hidden_states.reshape(N, H)
    q/k/v = x @ W{q,k,v}.T + b
    src, tgt = sort(edges_src), sort(edges_tgt)        # [E] each
    scores = softmax((q[tgt] @ k[src].T) / sqrt(HEAD), axis=0)   # over tgt axis
    v[tgt] = scores @ v[src]
    return v.reshape(B, S, H)

Sharding (8 cores):
  - node rows split 4096/core for the V linear (data-parallel, weights replicated)
  - tgt rows of the E x E score matrix split 1024/core
  - v[src] is computed DISTRIBUTED: each core projects its own 1024 src rows and
    one bf16 AllGather (8 MB) replicates the full v_src; launched right at kernel
    start so it hides under the scores loop
  - the k projection is eliminated by folding W2 = Wq^T Wk on the host; ALL bias
    terms fold exactly via x~_src = x_src + Wk^{-1} bk and b2 = bq @ Wk:
        s = (x_tgt W2 + b2) . x~_src / sqrt(HEAD)
    (identity is exact, incl. the bq.bk constant), so exp() writes straight into
    the SBUF-resident e matrix with no per-row rescale
  - softmax normalizer (per-src-column sum over the sharded tgt axis): two
    [128, 32] f32 AllReduces (first half mid-loop, second at loop end), queued
    behind the AllGather; pass 2 consumes jt in ascending order so the second
    AllReduce hides under phase E + the first half of pass 2
  - exp-scores (16 MB bf16/core) stay RESIDENT IN SBUF between the normalizer
    pass and the output matmul (no DRAM spill)
"""

import sys

sys.path.insert(0, "/opt/trn_rl_repo")

import numpy as np
from contextlib import ExitStack

import concourse.bass as bass
import concourse.bacc as bacc
import concourse.mybir as mybir
from concourse.tile import TileContext
from concourse.tile_rust import add_dep_helper
from concourse.bass_utils import run_bass_kernel_spmd

F32 = mybir.dt.float32
F32R = mybir.dt.float32r
BF16 = mybir.dt.bfloat16
AF = mybir.ActivationFunctionType

# problem constants
N_CORES = 8
B, S, H, NH = 4, 8192, 512, 8
HEAD = H // NH          # 64
N = B * S               # 32768
E = 8192
P = 128
FREE = 512              # matmul moving free dim (1 psum bank of f32)

N_OWN = N // N_CORES    # 4096 node rows per core
N_TGT = E // N_CORES    # 1024 tgt score rows per core

LAST_RESULT = None      # BassKernelResults of the most recent run (for test harness)
_PROGRAM = None


def dedup_ldweights(nc):
    """Remove InstLdweights that reload the stationary already in the PE array.

    The tile legalizer emits one InstLdweights per InstMatmult; for back-to-back
    matmuls sharing the same stationary slice the second (re)load costs a full
    ~128-cycle array drain + reload.  Drop it: the matmul (ldweights=False)
    streams against the already-loaded weights.  Any dependency edges of / onto
    the removed load are folded into the matmul that follows it.
    """
    renames = {}
    for fn in nc.m.functions:
        for blk in fn.blocks:
            insts = list(blk.instructions)
            new = []
            last_sig = None
            last_kept = None
            pend_del = None     # deleted LDW awaiting its matmul
            for inst in insts:
                tn = type(inst).__name__
                if tn == "InstLdweights":
                    sig = (inst.ins[0].concise(), inst.perf_mode,
                           inst.is_transpose, str(inst.tile_position),
                           str(inst.tile_size))
                    if sig == last_sig and last_kept is not None:
                        pend_del = inst
                        continue
                    last_sig = sig
                    last_kept = inst
                    new.append(inst)
                elif tn == "InstMatmult":
                    if pend_del is not None:
                        inst.merge_dependencies_from(pend_del)
                        renames[pend_del.name] = inst.name
                        pend_del = None
                    new.append(inst)
                else:
                    assert pend_del is None
                    last_sig = None
                    last_kept = None
                    new.append(inst)
            assert pend_del is None
            if len(new) != len(insts):
                blk.instructions = new
    if renames:
        for fn in nc.m.functions:
            for blk in fn.blocks:
                for inst in blk.instructions:
                    inst.remap_dependency_names(renames)
    return len(renames)


def build_program(h=H, e=E, n_own=N_OWN, n_tgt=N_TGT, n_cores=N_CORES, jblk=512):
    """Build the SPMD Bass program. All sizes in elements; h % 128 == 0,
    e % jblk == 0, jblk % 128 == 0, n_own % FREE == 0."""
    ft_n = h // P           # feature tiles
    jt_n = e // P           # src row tiles
    njb = e // jblk         # j blocks in the A/B loop
    j4_n = jblk // P        # 128-row tiles per j block
    ic_n = max(1, n_tgt // FREE)   # i chunks (tgt) per matmul pass
    icf = min(FREE, n_tgt)         # i chunk free size
    oc_n = max(1, n_own // FREE)
    ocf = min(FREE, n_own)
    jt_own = n_tgt // P            # src tiles this core projects for v_src
    inv_sqrt_head = 1.0 / np.sqrt(HEAD)
    jt_half = jt_n // 2

    nc = bacc.Bacc(num_devices=n_cores)

    xT_own = nc.declare_dram_parameter("xT_own", [h, n_own], F32R, isOutput=False)
    xT_src = nc.declare_dram_parameter("xT_src", [h, e], BF16, isOutput=False)
    xT_vso = nc.declare_dram_parameter("xT_vso", [h, n_tgt], BF16, isOutput=False)
    xT_tgt = nc.declare_dram_parameter("xT_tgt", [h, n_tgt], F32R, isOutput=False)
    w2T = nc.declare_dram_parameter("w2T", [h, h], F32R, isOutput=False)
    wvT = nc.declare_dram_parameter("wvT", [h, h], F32R, isOutput=False)
    wvbT = nc.declare_dram_parameter("wvbT", [h, h], BF16, isOutput=False)
    b2_t = nc.declare_dram_parameter("b2_t", [P, ft_n], F32, isOutput=False)
    bv_bc = nc.declare_dram_parameter("bv_bc", [P, h], F32, isOutput=False)
    v_own = nc.declare_dram_parameter("v_own", [n_own, h], F32, isOutput=True)
    outT_tgt = nc.declare_dram_parameter("outT_tgt", [h, n_tgt], F32, isOutput=True)

    cc_in_v = nc.dram_tensor("cc_in_v", [jt_own, P, h], BF16)
    cc_out_v = nc.dram_tensor("cc_out_v", [jt_n, P, h], BF16, addr_space="Shared")
    cc_in_a = nc.dram_tensor("cc_in_a", [P, jt_half], F32)
    cc_out_a = nc.dram_tensor("cc_out_a", [P, jt_half], F32, addr_space="Shared")
    cc_in_b = nc.dram_tensor("cc_in_b", [P, jt_n - jt_half], F32)
    cc_out_b = nc.dram_tensor("cc_out_b", [P, jt_n - jt_half], F32, addr_space="Shared")

    with TileContext(nc) as tc, ExitStack() as ctx:
        persist = ctx.enter_context(tc.tile_pool(name="persist", bufs=1))

        # ---- persistent SBUF state ----
        # e matrix lives in SBUF for the whole kernel: [P, jt_n, n_tgt] bf16
        # = 128 KiB/partition.
        e_sb = persist.tile([P, jt_n, n_tgt], BF16)
        q_sb = persist.tile([P, ft_n, n_tgt], BF16)
        wv_sb = persist.tile([P, ft_n, h], F32R)
        wvb_sb = persist.tile([P, ft_n, h], BF16)
        bvb_sb = persist.tile([P, h], F32)
        bq_sb = persist.tile([P, ft_n], F32)
        colsum_a = persist.tile([P, jt_half], F32)
        colsum_b = persist.tile([P, jt_n - jt_half], F32)
        csg_sb = persist.tile([P, jt_n], F32)
        recip_sb = persist.tile([P, jt_n], F32)

        # phase-VS loads first (vs_own -> AllGather is the long-latency path),
        # split per feature-subtile so the first matmul starts on slab 0;
        # then phase-Q loads, then the bulk streams
        wvbT_t = wvbT.rearrange("(ft p) f -> ft p f", p=P)
        for fs in range(ft_n):
            nc.sync.dma_start(wvb_sb[:, fs, :], wvbT_t[fs])
        nc.sync.dma_start(bvb_sb[:], bv_bc[:])
        nc.sync.dma_start(bq_sb[:], b2_t[:])

        xsp = ctx.enter_context(tc.tile_pool(name="xs", bufs=3))
        with (
            tc.tile_pool(name="qload", bufs=1) as qload,
            tc.tile_pool(name="vsp", bufs=1) as vsp,
            tc.tile_pool(name="vst", bufs=2) as vstp,
            tc.tile_pool(name="psvs", bufs=2, space="PSUM") as psvs,
            tc.tile_pool(name="psq", bufs=2, space="PSUM") as psq,
        ):
            xvo = vsp.tile([P, ft_n, n_tgt], BF16)
            xvoT_t = xT_vso.rearrange("(fs p) j -> fs p j", p=P)
            for fs in range(ft_n):
                nc.sync.dma_start(xvo[:, fs, :], xvoT_t[fs])
            wq_f, xtg_f = [], []
            for fs in range(ft_n):
                wqf = qload.tile([P, h], F32R, tag=f"wq{fs}", name=f"wq{fs}")
                nc.sync.dma_start(wqf[:], w2T[fs * P:(fs + 1) * P, :])
                xtf = qload.tile([P, n_tgt], F32R, tag=f"xtg{fs}", name=f"xtg{fs}")
                nc.sync.dma_start(xtf[:], xT_tgt[fs * P:(fs + 1) * P, :])
                wq_f.append(wqf)
                xtg_f.append(xtf)
            # first src block + Wv for phase E, behind the critical loads
            xs0 = xsp.tile([P, ft_n, jblk], BF16, tag="xs", name="xs0")
            nc.sync.dma_start(
                xs0[:],
                xT_src[:, 0:jblk].rearrange("(fs p) j -> p fs j", p=P),
            )
            nc.sync.dma_start(wv_sb[:], wvT.rearrange("(ft p) f -> p ft f", p=P))

            # ---- phase VS: vs_own = x_srcown @ Wv.T + bv, j-major tiles ----
            vs_stores = []
            for t in range(jt_own):
                pv = psvs.tile([P, h], F32)
                for fs in range(ft_n):
                    nc.tensor.matmul(
                        pv[:],
                        xvo[:, fs, t * P:(t + 1) * P],
                        wvb_sb[:, fs, :],
                        start=(fs == 0), stop=(fs == ft_n - 1),
                    )
                vt = vstp.tile([P, h], BF16)
                nc.vector.tensor_add(vt[:], pv[:], bvb_sb[:])
                vs_stores.append(nc.sync.dma_start(cc_in_v[t], vt[:]))

            # AllGather of v_src (8 MB bf16), hidden under phases Q/A/B/E
            ccv = nc.gpsimd.collective_compute(
                "AllGather", mybir.AluOpType.bypass,
                replica_groups=[list(range(n_cores))],
                ins=[cc_in_v[:]], outs=[cc_out_v[:]],
            )
            for st in vs_stores:
                add_dep_helper(ccv.ins, st.ins, sync=True,
                               reason="vs_own stores before allgather")

            # ---- phase Q: q_tgt^T = W2^T-matmul + bias, [h, n_tgt] f-major ----
            # fs outer / ic inner: back-to-back matmuls share the stationary
            # (the redundant reload is stripped by dedup_ldweights)
            for ftile in range(ft_n):
                pq = [psq.tile([P, icf], F32, tag=f"pq{ic}", name=f"pq{ic}")
                      for ic in range(ic_n)]
                for fs in range(ft_n):
                    for ic in range(ic_n):
                        nc.tensor.matmul(
                            pq[ic][:],
                            wq_f[fs][:, ftile * P:(ftile + 1) * P],
                            xtg_f[fs][:, ic * icf:(ic + 1) * icf],
                            start=(fs == 0), stop=(fs == ft_n - 1),
                        )
                for ic in range(ic_n):
                    nc.scalar.activation(
                        q_sb[:, ftile, ic * icf:(ic + 1) * icf], pq[ic][:],
                        AF.Identity, bias=bq_sb[:, ftile:ftile + 1],
                    )

        # ---- A/B loop: exp-scores into SBUF + colsum ----
        xop = ctx.enter_context(tc.tile_pool(name="xo", bufs=4))
        xo_tiles = {}

        def xo_load(oc):
            xo = xop.tile([P, ft_n, ocf], F32R, tag="xo")
            nc.sync.dma_start(
                xo[:],
                xT_own[:, oc * ocf:(oc + 1) * ocf].rearrange(
                    "(fs p) o -> p fs o", p=P),
            )
            xo_tiles[oc] = xo

        with tc.tile_pool(name="pss", bufs=3, space="PSUM") as pss:
            for jb in range(njb):
                if jb == 0:
                    xs = xs0
                else:
                    xs = xsp.tile([P, ft_n, jblk], BF16, tag="xs")
                    nc.sync.dma_start(
                        xs[:],
                        xT_src[:, jb * jblk:(jb + 1) * jblk].rearrange(
                            "(fs p) j -> p fs j", p=P),
                    )
                # scores^T via folded weights: s^T[j,i] = x~_src[j,:].q2[i,:]
                # e = exp(s/8) straight into SBUF; colsum = sum_i e
                for j4 in range(j4_n):
                    jt = jb * j4_n + j4
                    ps = pss.tile([P, n_tgt], F32)
                    for fs in range(ft_n):
                        for ic in range(ic_n):
                            nc.tensor.matmul(
                                ps[:, ic * icf:(ic + 1) * icf],
                                xs[:, fs, j4 * P:(j4 + 1) * P],
                                q_sb[:, fs, ic * icf:(ic + 1) * icf],
                                start=(fs == 0), stop=(fs == ft_n - 1),
                            )
                    if jt < jt_half:
                        acc = colsum_a[:, jt:jt + 1]
                    else:
                        acc = colsum_b[:, jt - jt_half:jt - jt_half + 1]
                    # Exp psum->SBUF bf16; colsum rides along via accum_out
                    nc.scalar.activation(
                        e_sb[:, jt, :], ps[:],
                        AF.Exp, scale=float(inv_sqrt_head),
                        accum_out=acc,
                    )

                if njb - 5 <= jb <= njb - 2:
                    xo_load(jb - (njb - 5))    # prefetch xo chunks 0..3

                if jb == max(njb // 2 - 1, 0):
                    # first-half colsum AllReduce, hidden under remaining A/B work
                    d1a = nc.sync.dma_start(cc_in_a[:], colsum_a[:])
                    cca = nc.gpsimd.collective_compute(
                        "AllReduce", mybir.AluOpType.add,
                        replica_groups=[list(range(n_cores))],
                        ins=[cc_in_a[:]], outs=[cc_out_a[:]],
                    )
                    add_dep_helper(cca.ins, d1a.ins, sync=True,
                                   reason="colsum_a store before allreduce")
                    d2a = nc.sync.dma_start(csg_sb[:, :jt_half], cc_out_a[:])
                    add_dep_helper(d2a.ins, cca.ins, sync=True,
                                   reason="allreduce_a before readback")
                    nc.vector.reciprocal(recip_sb[:, :jt_half],
                                         csg_sb[:, :jt_half])

        # ---- second-half colsum AllReduce (launch only; the readback +
        # reciprocal are emitted after phase E so the Vector FIFO is not
        # blocked waiting on the collective) ----
        d1b = nc.sync.dma_start(cc_in_b[:], colsum_b[:])
        ccb = nc.gpsimd.collective_compute(
            "AllReduce", mybir.AluOpType.add,
            replica_groups=[list(range(n_cores))],
            ins=[cc_in_b[:]], outs=[cc_out_b[:]],
        )
        add_dep_helper(ccb.ins, d1b.ins, sync=True,
                       reason="colsum_b store before allreduce")

        # ---- phase E: v_own = x_own @ Wv.T + bv (covers collective latency) ----
        with (
            tc.tile_pool(name="vo", bufs=3) as vop,
            tc.tile_pool(name="pse", bufs=2, space="PSUM") as pse,
        ):
            v_own_t = v_own.rearrange("(ot p) f -> ot p f", p=P)
            for oc in range(oc_n):
                if oc + 4 < oc_n:
                    xo_load(oc + 4)
                xo = xo_tiles.pop(oc)
                for o4 in range(ocf // P):
                    pe_ = pse.tile([P, h], F32)
                    for fs in range(ft_n):
                        nc.tensor.matmul(
                            pe_[:],
                            xo[:, fs, o4 * P:(o4 + 1) * P],
                            wv_sb[:, fs, :],
                            start=(fs == 0), stop=(fs == ft_n - 1),
                        )
                    vo = vop.tile([P, h], F32)
                    nc.vector.tensor_add(vo[:], pe_[:], bvb_sb[:])
                    nc.sync.dma_start(v_own_t[oc * (ocf // P) + o4], vo[:])

        # ---- pass 2: out^T += (v_src/colsum)^T-matmul over SBUF e ----
        with (
            tc.tile_pool(name="cv", bufs=4) as cvp,
            tc.tile_pool(name="co", bufs=2) as cop,
            tc.tile_pool(name="psc", bufs=1, space="PSUM") as pscp,
        ):
            psc_f = [pscp.tile([P, n_tgt], F32, tag=f"psc{f}", name=f"psc{f}")
                     for f in range(ft_n)]
            for jt in range(jt_n):
                if jt == jt_half:
                    # colsum_b readback + reciprocal, emitted here so no
                    # engine FIFO ahead of the first jt_half iterations can
                    # block on the collective
                    d2b = nc.sync.dma_start(csg_sb[:, jt_half:], cc_out_b[:])
                    add_dep_helper(d2b.ins, ccb.ins, sync=True,
                                   reason="allreduce_b before readback")
                    nc.vector.reciprocal(recip_sb[:, jt_half:],
                                         csg_sb[:, jt_half:])
                vt = cvp.tile([P, h], BF16)
                dvt = nc.sync.dma_start(vt[:], cc_out_v[jt])
                add_dep_helper(dvt.ins, ccv.ins, sync=True,
                               reason="v_src allgather before readback")
                nc.vector.tensor_scalar_mul(vt[:], vt[:], recip_sb[:, jt:jt + 1])
                for ftile in range(ft_n):
                    for ic in range(ic_n):
                        nc.tensor.matmul(
                            psc_f[ftile][:, ic * icf:(ic + 1) * icf],
                            vt[:, ftile * P:(ftile + 1) * P],
                            e_sb[:, jt, ic * icf:(ic + 1) * icf],
                            start=(jt == 0), stop=(jt == jt_n - 1),
                        )
            for ftile in range(ft_n):
                ot = cop.tile([P, n_tgt], F32)
                nc.vector.tensor_copy(ot[:], psc_f[ftile][:])
                nc.sync.dma_start(outT_tgt[ftile * P:(ftile + 1) * P, :], ot[:])

    n_dedup = dedup_ldweights(nc)
    assert n_dedup > 0, "ldweights dedup pass matched nothing"
    nc.compile()
    return nc


def _get_program():
    global _PROGRAM
    if _PROGRAM is None:
        _PROGRAM = build_program()
    return _PROGRAM


def make_in_maps(hidden_states, Wq, bq, Wk, bk, Wv, bv, edges_src, edges_tgt,
                 h=H, e=E, n_own=N_OWN, n_tgt=N_TGT, n_cores=N_CORES):
    """Host-side sharding: sort indices, gather rows, transpose to f-major."""
    ft_n = h // P
    n = n_own * n_cores
    x = np.ascontiguousarray(
        np.asarray(hidden_states, dtype=np.float32).reshape(n, h))
    src = np.sort(np.asarray(edges_src).astype(np.int64))
    tgt = np.sort(np.asarray(edges_tgt).astype(np.int64))
    xT = np.ascontiguousarray(x.T)                      # [h, n]
    import ml_dtypes
    # weight folding: s = q @ k_src^T = (x_tgt W2 + bq Wk) . (x_src + Wk^-1 bk)
    # -- exact, absorbs every bias term (incl. the bq.bk constant)
    Wq64 = np.asarray(Wq, np.float64)
    Wk64 = np.asarray(Wk, np.float64)
    bq64 = np.asarray(bq, np.float64)
    bk64 = np.asarray(bk, np.float64)
    w2T = np.ascontiguousarray((Wq64.T @ Wk64).astype(np.float32))
    b2 = (bq64 @ Wk64).astype(np.float32)
    shift = np.linalg.solve(Wk64, bk64)
    xT_src_f = xT[:, src].astype(np.float64) + shift[:, None]
    xT_src = np.ascontiguousarray(
        xT_src_f.astype(ml_dtypes.bfloat16))            # [h, e] bf16, shifted
    xT_vso_full = xT[:, src]                            # unshifted, for v_src
    wvT = np.ascontiguousarray(np.asarray(Wv, np.float32).T)
    wvbT = np.ascontiguousarray(wvT.astype(ml_dtypes.bfloat16))
    b2_t = np.ascontiguousarray(b2.reshape(ft_n, P).T)
    bv_bc = np.ascontiguousarray(
        np.tile(np.asarray(bv, np.float32)[None, :], (P, 1)))
    in_maps = []
    for c in range(n_cores):
        in_maps.append({
            "xT_own": np.ascontiguousarray(xT[:, c * n_own:(c + 1) * n_own]),
            "xT_src": xT_src,
            "xT_vso": np.ascontiguousarray(
                xT_vso_full[:, c * n_tgt:(c + 1) * n_tgt].astype(
                    ml_dtypes.bfloat16)),
            "xT_tgt": np.ascontiguousarray(xT[:, tgt[c * n_tgt:(c + 1) * n_tgt]]),
            "w2T": w2T, "wvT": wvT, "wvbT": wvbT,
            "b2_t": b2_t, "bv_bc": bv_bc,
        })
    return in_maps, tgt


def assemble_output(results, tgt, h=H, n_own=N_OWN, n_tgt=N_TGT,
                    n_cores=N_CORES, out_shape=(B, S, H)):
    n = n_own * n_cores
    v = np.empty((n, h), np.float32)
    for c in range(n_cores):
        v[c * n_own:(c + 1) * n_own] = results[c]["v_own"]
    outs = np.concatenate(
        [results[c]["outT_tgt"].T for c in range(n_cores)], axis=0)
    v[tgt] = outs
    return v.reshape(out_shape)


def kernel(hidden_states, Wq, bq, Wk, bk, Wv, bv, edges_src, edges_tgt):
    global LAST_RESULT
    in_maps, tgt = make_in_maps(
        hidden_states, Wq, bq, Wk, bk, Wv, bv, edges_src, edges_tgt)
    nc = _get_program()
    res = run_bass_kernel_spmd(nc, in_maps, list(range(N_CORES)))
    LAST_RESULT = res
    return assemble_output(res.results, tgt)



# revision 18
# speedup vs baseline: 1.5964x; 1.5964x over previous
"""Trainium2 Bass kernel for nn_DGraphAttention (gnn_message_passing).

Math (reference):
    x = hidden_states.reshape(N, H)
    q/k/v = x @ W{q,k,v}.T + b
    src, tgt = sort(edges_src), sort(edges_tgt)        # [E] each
    scores = softmax((q[tgt] @ k[src].T) / sqrt(HEAD), axis=0)   # over tgt axis
    v[tgt] = scores @ v[src]
    return v.reshape(B, S, H)

Sharding (8 cores):
  - node rows split 4096/core for the V linear (data-parallel, weights
    replicated); tgt rows of the E x E score matrix split 1024/core
  - v[src] computed DISTRIBUTED: each core projects its 1024 src rows; one fp8
    AllGather (4 MB) replicates the full v_src
  - softmax normalizer: each core AllGathers its PARTIAL colsum (sum over its
    1024 tgt rows, [128,64] f32) and sums the 8 partials locally -- one small
    AllGather instead of two AllReduces (the collectives stream is strictly
    serial with 15-30us latency per op, so fewer+earlier ops win)
  - k projection eliminated by folding W2 = Wq^T Wk on the host; all bias
    terms fold exactly via x~_src = x_src + Wk^{-1} bk and b2 = bq @ Wk;
    v_src reuses x~_src with bias bv' = bv - (Wk^{-1} bk) @ Wv^T (exact)

fp8 plan (validated vs reference on CPU, global rel_l2 ~1e-3):
  - the two big E x E matmuls (scores, scores @ v_src) run in fp8e4 with
    MatmulPerfMode.DoubleRow: 2 contraction k-tiles per instruction
  - e = exp(s/sqrt(HEAD) - 2) stored fp8 in SBUF (the -2 shift cancels in the
    column softmax and keeps e inside e4m3 range)
  - vt_scaled = fp8(v_src * 4096/colsum): the 4096 factor keeps values above
    the e4m3 subnormal floor; the final PSUM->SBUF copy divides by 4096
  - v_own (the untouched rows, 97% of the output norm) runs in bf16
  - colsum on the Vector engine (tensor_reduce over fp8 e tiles), keeping the
    Scalar engine free for the 64 Exp activations

Schedule: phase E (v_own) is interleaved into the A/B score loop (the loop is
Scalar/Vector-paced, so the Tensor engine absorbs E for free), and pass 2 is
pr-outer so it starts the moment the colsum AllGather lands.
"""

import sys

sys.path.insert(0, "/opt/trn_rl_repo")

import numpy as np
from contextlib import ExitStack

import concourse.bass as bass
import concourse.bacc as bacc
import concourse.mybir as mybir
from concourse.tile import TileContext
from concourse.tile_rust import add_dep_helper
from concourse.bass_utils import run_bass_kernel_spmd

F32 = mybir.dt.float32
F32R = mybir.dt.float32r
BF16 = mybir.dt.bfloat16
FP8 = mybir.dt.float8e4
AF = mybir.ActivationFunctionType
DR = mybir.MatmulPerfMode.DoubleRow

# problem constants
N_CORES = 8
B, S, H, NH = 4, 8192, 512, 8
HEAD = H // NH          # 64
N = B * S               # 32768
E = 8192
P = 128
FREE = 512              # matmul moving free dim (1 psum bank of f32)

N_OWN = N // N_CORES    # 4096 node rows per core
N_TGT = E // N_CORES    # 1024 tgt score rows per core

SC = 4096.0             # fp8 underflow guard on vt_scaled
EXPC = 2.0              # global exp shift (cancels in column softmax)

LAST_RESULT = None      # BassKernelResults of the most recent run (for harness)
_PROGRAM = None


def dedup_ldweights(nc):
    """Remove InstLdweights that reload the stationary already in the PE array.

    The tile legalizer emits one InstLdweights per InstMatmult; for
    back-to-back matmuls sharing the same stationary slice the reload costs a
    full array drain + reload.  Drop it; dependency edges fold into the
    following matmul.
    """
    renames = {}
    for fn in nc.m.functions:
        for blk in fn.blocks:
            insts = list(blk.instructions)
            new = []
            last_sig = None
            last_kept = None
            pend_del = None     # deleted LDW awaiting its matmul
            for inst in insts:
                tn = type(inst).__name__
                if tn == "InstLdweights":
                    sig = (inst.ins[0].concise(), inst.perf_mode,
                           inst.is_transpose, str(inst.tile_position),
                           str(inst.tile_size))
                    if sig == last_sig and last_kept is not None:
                        pend_del = inst
                        continue
                    last_sig = sig
                    last_kept = inst
                    new.append(inst)
                elif tn == "InstMatmult":
                    if pend_del is not None:
                        inst.merge_dependencies_from(pend_del)
                        renames[pend_del.name] = inst.name
                        pend_del = None
                    new.append(inst)
                else:
                    assert pend_del is None
                    last_sig = None
                    last_kept = None
                    new.append(inst)
            assert pend_del is None
            if len(new) != len(insts):
                blk.instructions = new
    if renames:
        for fn in nc.m.functions:
            for blk in fn.blocks:
                for inst in blk.instructions:
                    inst.remap_dependency_names(renames)
    return len(renames)


def build_program(h=H, e=E, n_own=N_OWN, n_tgt=N_TGT, n_cores=N_CORES,
                  jblk=1024):
    ft_n = h // P           # feature tiles (4)
    fp_n = ft_n // 2        # feature-tile PAIRS for DoubleRow (2)
    jt_n = e // P           # src row tiles (64)
    njb = e // jblk         # j blocks in the A/B loop (8)
    j4_n = jblk // P        # 128-row tiles per j block (8)
    pr_n = jt_n // 2        # src row-tile pairs for pass 2 (32)
    ic_n = n_tgt // FREE    # i chunks (tgt) per matmul pass (2)
    oc_n = n_own // FREE    # own chunks for phase E (8)
    o4_n = FREE // P        # 128-row tiles per own chunk (4)
    eg_n = oc_n * o4_n      # E-phase matmul groups (32)
    jt_own = n_tgt // P     # src tiles this core projects for v_src (8)
    inv_sqrt_head = 1.0 / np.sqrt(HEAD)

    nc = bacc.Bacc(num_devices=n_cores)

    xT_src = nc.declare_dram_parameter("xT_src", [h, e], FP8, isOutput=False)
    xT_vso = nc.declare_dram_parameter("xT_vso", [h, n_tgt], FP8, isOutput=False)
    xT_tgt = nc.declare_dram_parameter("xT_tgt", [h, n_tgt], BF16, isOutput=False)
    xT_own = nc.declare_dram_parameter("xT_own", [h, n_own], BF16, isOutput=False)
    w2T = nc.declare_dram_parameter("w2T", [h, h], BF16, isOutput=False)
    wvT = nc.declare_dram_parameter("wvT", [h, h], BF16, isOutput=False)
    wv8T = nc.declare_dram_parameter("wv8T", [h, h], FP8, isOutput=False)
    b2_t = nc.declare_dram_parameter("b2_t", [P, ft_n], F32, isOutput=False)
    bv_bc = nc.declare_dram_parameter("bv_bc", [P, h], F32, isOutput=False)
    bvp_bc = nc.declare_dram_parameter("bvp_bc", [P, h], F32, isOutput=False)
    v_own = nc.declare_dram_parameter("v_own", [n_own, h], F32, isOutput=True)
    outT_tgt = nc.declare_dram_parameter("outT_tgt", [h, n_tgt], F32, isOutput=True)

    cc_in_v = nc.dram_tensor("cc_in_v", [jt_own, P, h], FP8)
    cc_out_v = nc.dram_tensor("cc_out_v", [jt_n, P, h], FP8, addr_space="Shared")
    cc_in_c = nc.dram_tensor("cc_in_c", [P, jt_n], F32)
    cc_out_c = nc.dram_tensor("cc_out_c", [n_cores, P, jt_n], F32,
                              addr_space="Shared")

    with TileContext(nc) as tc, ExitStack() as ctx:
        persist = ctx.enter_context(tc.tile_pool(name="persist", bufs=1))

        # ---- persistent SBUF state ----
        e_sb = persist.tile([P, jt_n, n_tgt], FP8)       # 64 KiB/part
        q_sb = persist.tile([P, ft_n, n_tgt], FP8)       # 4 KiB
        vts = persist.tile([P, pr_n, 2, h], FP8)         # 32 KiB (scaled v_src)
        wv_sb = persist.tile([P, ft_n, h], BF16)         # 4 KiB
        wvb_sb = persist.tile([P, ft_n, h], FP8)         # 2 KiB
        bvb_sb = persist.tile([P, h], F32)
        bvp_sb = persist.tile([P, h], F32)
        bq_sb = persist.tile([P, ft_n], F32)
        colsum_sb = persist.tile([P, jt_n], F32)
        csg_sb = persist.tile([P, jt_n], F32)
        recip_sb = persist.tile([P, jt_n], F32)          # 4096/colsum
        expc_sb = persist.tile([P, 1], F32)              # exp shift constant
        nc.gpsimd.memset(expc_sb[:], -float(EXPC))

        # ---- phase-VS-critical loads first (AllGather is the long pole) ----
        nc.sync.dma_start(wvb_sb[:], wv8T.rearrange("(ft p) f -> p ft f", p=P))
        nc.sync.dma_start(bvp_sb[:], bvp_bc[:])

        xsp = ctx.enter_context(tc.tile_pool(name="xs", bufs=2))
        with (
            tc.tile_pool(name="qload", bufs=1) as qload,
            tc.tile_pool(name="vsp", bufs=1) as vsp,
            tc.tile_pool(name="vst", bufs=2) as vstp,
            tc.tile_pool(name="psvs", bufs=2, space="PSUM") as psvs,
            tc.tile_pool(name="psq", bufs=2, space="PSUM") as psq,
        ):
            xvo = vsp.tile([P, ft_n, n_tgt], FP8)
            nc.sync.dma_start(xvo[:], xT_vso.rearrange("(fs p) j -> p fs j", p=P))
            # Q loads on the Scalar queue (frees SP for the VS-critical path)
            wq_f, xtg_f = [], []
            for fs in range(ft_n):
                wqf = qload.tile([P, h], BF16, tag=f"wq{fs}", name=f"wq{fs}")
                nc.scalar.dma_start(wqf[:], w2T[fs * P:(fs + 1) * P, :])
                xtf = qload.tile([P, n_tgt], BF16, tag=f"xtg{fs}", name=f"xtg{fs}")
                nc.scalar.dma_start(xtf[:], xT_tgt[fs * P:(fs + 1) * P, :])
                wq_f.append(wqf)
                xtg_f.append(xtf)
            nc.sync.dma_start(bq_sb[:], b2_t[:])
            nc.sync.dma_start(bvb_sb[:], bv_bc[:])
            # first src block + Wv for phase E, behind the critical loads
            xs0 = xsp.tile([P, ft_n, jblk], FP8, tag="xs", name="xs0")
            nc.sync.dma_start(
                xs0[:],
                xT_src[:, 0:jblk].rearrange("(fs p) j -> p fs j", p=P),
            )
            nc.sync.dma_start(wv_sb[:], wvT.rearrange("(ft p) f -> p ft f", p=P))

            # ---- phase VS: vs_own = x~_srcown @ Wv.T + bv' (fp8 DoubleRow) ----
            vs_stores = []
            for t in range(jt_own):
                pv = psvs.tile([P, h], F32)
                for fp in range(fp_n):
                    nc.tensor.matmul(
                        pv[:],
                        xvo[:, 2 * fp:2 * fp + 2, t * P:(t + 1) * P],
                        wvb_sb[:, 2 * fp:2 * fp + 2, :],
                        start=(fp == 0), stop=(fp == fp_n - 1),
                        perf_mode=DR,
                    )
                vt = vstp.tile([P, h], FP8)
                nc.vector.tensor_add(vt[:], pv[:], bvp_sb[:])
                vs_stores.append(nc.sync.dma_start(cc_in_v[t], vt[:]))

            # AllGather of v_src (4 MB fp8)
            ccv = nc.gpsimd.collective_compute(
                "AllGather", mybir.AluOpType.bypass,
                replica_groups=[list(range(n_cores))],
                ins=[cc_in_v[:]], outs=[cc_out_v[:]],
            )
            for st in vs_stores:
                add_dep_helper(ccv.ins, st.ins, sync=True,
                               reason="vs_own stores before allgather")

            # ---- phase Q: q2^T = W2^T-matmul + bias (bf16 in, fp8 out) ----
            for ftile in range(ft_n):
                pq = [psq.tile([P, FREE], F32, tag=f"pq{ic}", name=f"pq{ic}")
                      for ic in range(ic_n)]
                for fs in range(ft_n):
                    for ic in range(ic_n):
                        nc.tensor.matmul(
                            pq[ic][:],
                            wq_f[fs][:, ftile * P:(ftile + 1) * P],
                            xtg_f[fs][:, ic * FREE:(ic + 1) * FREE],
                            start=(fs == 0), stop=(fs == ft_n - 1),
                        )
                for ic in range(ic_n):
                    nc.scalar.activation(
                        q_sb[:, ftile, ic * FREE:(ic + 1) * FREE], pq[ic][:],
                        AF.Identity, bias=bq_sb[:, ftile:ftile + 1],
                    )

        # ---- merged A/B + phase-E loop ----
        # A/B: fp8 DR score matmuls -> Exp (Scalar) -> colsum (Vector).
        # The loop is Scalar/Vector-paced, so ~half of phase E's bf16 matmul
        # groups ride along on the idle Tensor engine; the rest run after.
        xop = ctx.enter_context(tc.tile_pool(name="xo", bufs=4))
        vop = ctx.enter_context(tc.tile_pool(name="vo", bufs=2))
        xo_tiles = {}

        def xo_load(oc):
            xo = xop.tile([P, ft_n, FREE], BF16, tag="xo")
            nc.sync.dma_start(
                xo[:],
                xT_own[:, oc * FREE:(oc + 1) * FREE].rearrange(
                    "(fs p) o -> p fs o", p=P),
            )
            xo_tiles[oc] = xo

        v_own_t = v_own.rearrange("(oc o4 p) f -> oc p o4 f", p=P, o4=o4_n)
        vo_state = {}

        def e_group(g, pse):
            oc, o4 = g // o4_n, g % o4_n
            if o4 == 0:
                vo_state[oc] = vop.tile([P, o4_n, h], F32, name=f"vo{oc}",
                                        tag="vo")
            vo = vo_state[oc]
            xo = xo_tiles[oc]
            pe_ = pse.tile([P, h], F32)
            for fs in range(ft_n):
                nc.tensor.matmul(
                    pe_[:],
                    xo[:, fs, o4 * P:(o4 + 1) * P],
                    wv_sb[:, fs, :],
                    start=(fs == 0), stop=(fs == ft_n - 1),
                )
            nc.vector.tensor_add(vo[:, o4, :], pe_[:], bvb_sb[:])
            if o4 == o4_n - 1:
                nc.sync.dma_start(v_own_t[oc], vo[:])
                xo_tiles.pop(oc)
                vo_state.pop(oc)

        with (
            tc.tile_pool(name="pss", bufs=3, space="PSUM") as pss,
            tc.tile_pool(name="pse", bufs=2, space="PSUM") as pse,
        ):
            for jb in range(njb):
                if jb == 0:
                    xs = xs0
                else:
                    xs = xsp.tile([P, ft_n, jblk], FP8, tag="xs")
                    nc.sync.dma_start(
                        xs[:],
                        xT_src[:, jb * jblk:(jb + 1) * jblk].rearrange(
                            "(fs p) j -> p fs j", p=P),
                    )
                for j4 in range(j4_n):
                    jt = jb * j4_n + j4
                    if jt % 12 == 0 and jt // 12 < 4:
                        xo_load(jt // 12)          # chunks 0..3
                    elif jt in (40, 48, 56):
                        xo_load(4 + (jt - 40) // 8)  # chunks 4..6
                    ps = pss.tile([P, n_tgt], F32)
                    for fp in range(fp_n):
                        for ic in range(ic_n):
                            nc.tensor.matmul(
                                ps[:, ic * FREE:(ic + 1) * FREE],
                                xs[:, 2 * fp:2 * fp + 2, j4 * P:(j4 + 1) * P],
                                q_sb[:, 2 * fp:2 * fp + 2,
                                     ic * FREE:(ic + 1) * FREE],
                                start=(fp == 0), stop=(fp == fp_n - 1),
                                perf_mode=DR,
                            )
                    # e = exp(s/8 - 2) straight to fp8 SBUF (Scalar engine)
                    nc.scalar.activation(
                        e_sb[:, jt, :], ps[:],
                        AF.Exp, scale=float(inv_sqrt_head),
                        bias=expc_sb[:, 0:1],
                    )
                    # colsum partial on the Vector engine from the fp8 e tile
                    nc.vector.tensor_reduce(
                        colsum_sb[:, jt:jt + 1], e_sb[:, jt, :],
                        mybir.AxisListType.X, mybir.AluOpType.add,
                    )
                    # interleave E groups 0..15 at jt = 8, 11, ..., 53
                    if jt >= 8 and (jt - 8) % 3 == 0 and (jt - 8) // 3 < 16:
                        e_group((jt - 8) // 3, pse)

            # colsum-partial AllGather (16 KB): replaces two AllReduces
            d1c = nc.sync.dma_start(cc_in_c[:], colsum_sb[:])
            cc2 = nc.gpsimd.collective_compute(
                "AllGather", mybir.AluOpType.bypass,
                replica_groups=[list(range(n_cores))],
                ins=[cc_in_c[:]], outs=[cc_out_c[:]],
            )
            add_dep_helper(cc2.ins, d1c.ins, sync=True,
                           reason="colsum store before allgather")

            # remaining E groups 16..31 (tensor-only tail of phase E)
            xo_load(7)
            for g in range(16, eg_n):
                e_group(g, pse)

        # vt raw tiles from the v_src AllGather, on the GpSimd queue
        with (
            tc.tile_pool(name="vr", bufs=1) as vrp,
            tc.tile_pool(name="cs8", bufs=1) as csp,
        ):
            vraw_tiles = {}

            def vr_load(blk):
                vr = vrp.tile([P, 8, h], FP8, tag=f"vr{blk % 4}",
                              name=f"vr{blk}")
                dv = nc.gpsimd.dma_start(
                    vr[:],
                    cc_out_v[blk * 8:(blk + 1) * 8].rearrange("t p f -> p t f"),
                )
                add_dep_helper(dv.ins, ccv.ins, sync=True,
                               reason="v_src allgather before readback")
                vraw_tiles[blk] = vr

            for blk in range(4):
                vr_load(blk)

            # colsum partials readback + local 8-way sum -> recip = 4096/colsum
            csg8 = csp.tile([P, n_cores, jt_n], F32)
            d2c = nc.sync.dma_start(
                csg8[:], cc_out_c.rearrange("c p t -> p c t"))
            add_dep_helper(d2c.ins, cc2.ins, sync=True,
                           reason="colsum allgather before readback")
            nc.vector.tensor_add(csg_sb[:], csg8[:, 0, :], csg8[:, 1, :])
            for c in range(2, n_cores):
                nc.vector.tensor_add(csg_sb[:], csg_sb[:], csg8[:, c, :])
            nc.vector.reciprocal(recip_sb[:], csg_sb[:])
            nc.vector.tensor_scalar_mul(recip_sb[:], recip_sb[:], SC)

            # ---- pass 2: out^T = (vts)^T-matmul over SBUF e, pr-outer so it
            # starts the moment recip lands; scales alternate Vector/Scalar ----
            def scale_vt(pr):
                for k in range(2):
                    jt = 2 * pr + k
                    src_ap = vraw_tiles[jt // 8][:, jt % 8, :]
                    dst_ap = vts[:, pr, k, :]
                    if pr % 2 == 0:
                        nc.vector.tensor_scalar_mul(
                            dst_ap, src_ap, recip_sb[:, jt:jt + 1])
                    else:
                        nc.scalar.activation(
                            dst_ap, src_ap, AF.Copy,
                            scale=recip_sb[:, jt:jt + 1])

            with (
                tc.tile_pool(name="co", bufs=2) as cop,
                tc.tile_pool(name="psc", bufs=1, space="PSUM") as pscp,
            ):
                psc_f = [pscp.tile([P, n_tgt], F32, tag=f"psc{f}",
                                   name=f"psc{f}")
                         for f in range(ft_n)]
                for pr in range(pr_n):
                    # vr blocks 4..7 reuse the buffers of 0..3; emit each load
                    # right after the reused buffer's last reader (WAR-safe)
                    if 4 <= pr <= 16 and pr % 4 == 0:
                        vr_load(pr // 4 + 3)
                    scale_vt(pr)
                    for ftile in range(ft_n):
                        for ic in range(ic_n):
                            nc.tensor.matmul(
                                psc_f[ftile][:, ic * FREE:(ic + 1) * FREE],
                                vts[:, pr, :, ftile * P:(ftile + 1) * P],
                                e_sb[:, 2 * pr:2 * pr + 2,
                                     ic * FREE:(ic + 1) * FREE],
                                start=(pr == 0), stop=(pr == pr_n - 1),
                                perf_mode=DR,
                            )
                for ftile in range(ft_n):
                    ot = cop.tile([P, n_tgt], F32)
                    # undo the 4096 vt guard in the PSUM->SBUF copy
                    nc.scalar.activation(ot[:], psc_f[ftile][:], AF.Copy,
                                         scale=float(1.0 / SC))
                    nc.sync.dma_start(outT_tgt[ftile * P:(ftile + 1) * P, :],
                                      ot[:])

    n_dedup = dedup_ldweights(nc)
    assert n_dedup > 0, "ldweights dedup pass matched nothing"
    nc.compile()
    return nc


def _get_program():
    global _PROGRAM
    if _PROGRAM is None:
        _PROGRAM = build_program()
    return _PROGRAM


def make_in_maps(hidden_states, Wq, bq, Wk, bk, Wv, bv, edges_src, edges_tgt,
                 h=H, e=E, n_own=N_OWN, n_tgt=N_TGT, n_cores=N_CORES):
    """Host-side sharding: sort indices, gather rows, fold weights, quantize."""
    ft_n = h // P
    n = n_own * n_cores
    import ml_dtypes
    E4 = ml_dtypes.float8_e4m3
    x = np.ascontiguousarray(
        np.asarray(hidden_states, dtype=np.float32).reshape(n, h))
    src = np.sort(np.asarray(edges_src).astype(np.int64))
    tgt = np.sort(np.asarray(edges_tgt).astype(np.int64))
    xT = np.ascontiguousarray(x.T)                      # [h, n]
    # weight folding: s = (x_tgt W2 + b2) . x~_src, x~ = x + Wk^-1 bk (exact)
    Wq64 = np.asarray(Wq, np.float64)
    Wk64 = np.asarray(Wk, np.float64)
    Wv64 = np.asarray(Wv, np.float64)
    bq64 = np.asarray(bq, np.float64)
    bk64 = np.asarray(bk, np.float64)
    bv64 = np.asarray(bv, np.float64)
    w2T = np.ascontiguousarray((Wq64.T @ Wk64).astype(ml_dtypes.bfloat16))
    b2 = (bq64 @ Wk64).astype(np.float32)
    shift = np.linalg.solve(Wk64, bk64)
    xT_src_f = xT[:, src].astype(np.float64) + shift[:, None]
    xT_src = np.ascontiguousarray(xT_src_f.astype(E4))  # [h, e] fp8, shifted
    wvT = np.ascontiguousarray(Wv64.T.astype(ml_dtypes.bfloat16))
    wv8T = np.ascontiguousarray(Wv64.T.astype(E4))
    bvp = (bv64 - shift @ Wv64.T).astype(np.float32)    # bias for shifted x~
    b2_t = np.ascontiguousarray(b2.reshape(ft_n, P).T)
    bv_bc = np.ascontiguousarray(
        np.tile(np.asarray(bv, np.float32)[None, :], (P, 1)))
    bvp_bc = np.ascontiguousarray(np.tile(bvp[None, :], (P, 1)))
    xtgT = xT[:, tgt]
    in_maps = []
    for c in range(n_cores):
        in_maps.append({
            "xT_src": xT_src,
            "xT_vso": np.ascontiguousarray(
                xT_src[:, c * n_tgt:(c + 1) * n_tgt]),
            "xT_tgt": np.ascontiguousarray(
                xtgT[:, c * n_tgt:(c + 1) * n_tgt].astype(ml_dtypes.bfloat16)),
            "xT_own": np.ascontiguousarray(
                xT[:, c * n_own:(c + 1) * n_own].astype(ml_dtypes.bfloat16)),
            "w2T": w2T, "wvT": wvT, "wv8T": wv8T,
            "b2_t": b2_t, "bv_bc": bv_bc, "bvp_bc": bvp_bc,
        })
    return in_maps, tgt


def assemble_output(results, tgt, h=H, n_own=N_OWN, n_tgt=N_TGT,
                    n_cores=N_CORES, out_shape=(B, S, H)):
    n = n_own * n_cores
    v = np.empty((n, h), np.float32)
    for c in range(n_cores):
        v[c * n_own:(c + 1) * n_own] = results[c]["v_own"]
    outs = np.concatenate(
        [results[c]["outT_tgt"].T for c in range(n_cores)], axis=0)
    v[tgt] = outs
    return v.reshape(out_shape)


def kernel(hidden_states, Wq, bq, Wk, bk, Wv, bv, edges_src, edges_tgt):
    global LAST_RESULT
    in_maps, tgt = make_in_maps(
        hidden_states, Wq, bq, Wk, bk, Wv, bv, edges_src, edges_tgt)
    nc = _get_program()
    res = run_bass_kernel_spmd(nc, in_maps, list(range(N_CORES)))
    LAST_RESULT = res
    return assemble_output(res.results, tgt)
